# revision 2
# baseline (speedup 1.0000x reference)
"""Trainium2 Bass kernel for nn_AggregatedBilinear.

Computation (per batch row b):
    x1 = ELU(input1 @ W1.T)                    # [128]
    x2 = ELU(input2 @ W2.T)                    # [128]
    y[g,o] = sum_ij x1[g,i] Wb[g,o,i,j] x2[g,j]   (g<32, o,i,j<4)
    out = ELU(y) @ Wout.T                      # [512]

Strategy: data-parallel over 8 NeuronCores (8192 batch rows each). On-chip
layout is feature-major (features on SBUF partitions, batch on the free dim),
so the host pre-transposes each input shard (fp32 DMA-transpose does not
exist on trn2) and transposes the result back.

The per-group bilinear is computed as
    t'[(g,i,j)] = (E1 @ A1') * (E2 @ A2')       # PE expand + DVE multiply
    Y = Wblk.T @ t' - C1 @ A1' - C2 @ A2'       # PE, block-diagonal consts
where A' = ELU(x)+1 = relu(x) + exp(min(x,0)) (computed with 2 ACT ops + one
fused DVE scalar_tensor_tensor), and the -1 corrections are folded into the
constant matrices C1[(g,o),(g,i)] = sum_j Wb[g,o,i,j], C2, and per-partition
bias vectors (c0 = sum_ij Wb, bout = -Wout @ (1 - c0)).
"""

import numpy as np

B = 65536
IN1 = IN2 = 512
OUT = 512
CARD = 32
WIDTH = 4
INTERNAL = CARD * WIDTH  # 128
N_CORES = 8
B_CORE = B // N_CORES  # 8192
NT = 512  # batch columns per tile (one PSUM bank)
N_TILES = B_CORE // NT  # 16

_CACHE = {}


def _ensure_path():
    import sys

    try:
        import concourse  # noqa: F401
    except ImportError:
        for p in ("/opt/trn_rl_repo", "/root/.axon_site/_ro/trn_rl_repo"):
            if p not in sys.path:
                sys.path.insert(0, p)


def _split_excess_waits(nc, max_waits=1):
    """walrus CoreV3 codegen rejects instructions with more than a couple of
    semaphore waits; split excess waits onto preceding NoOps."""
    from concourse import mybir

    n_new = 0
    for f in nc.m.functions:
        for bb in f.blocks:
            insts = list(bb.instructions)
            out = []
            changed = False
            for inst in insts:
                si = inst.sync_info
                if si is not None and si.on_wait and len(si.on_wait) > max_waits:
                    waits = list(si.on_wait)
                    excess, keep = waits[:-max_waits], waits[-max_waits:]
                    for i in range(0, len(excess), max_waits):
                        nop = mybir.InstNoOp(
                            name=f"waitsplit-{n_new}",
                            engine=inst.engine,
                            ins=[],
                            outs=[],
                            sync_info=mybir.SyncInfo(
                                on_wait=excess[i : i + max_waits], on_update=[]
                            ),
                        )
                        n_new += 1
                        out.append(nop)
                    inst.sync_info = mybir.SyncInfo(
                        on_wait=keep, on_update=list(si.on_update or [])
                    )
                    changed = True
                out.append(inst)
            if changed:
                bb.instructions[:] = out
    return n_new


DEFAULT_CFG = dict(db=2, bufs_in=2, bufs_pf=1, bufs_pr=1, bufs_py=2, bufs_po=1,
                   bufs_t=4, bufs_out=2)


def _build_program(mm_dtype_name="float32r", reps=1, loop_reps=None, cfg=None):
    """loop_reps: wrap the whole batch sweep in an on-device For_i that runs
    it loop_reps times (same I/O; used only for timing measurements)."""
    cfg = dict(DEFAULT_CFG, **(cfg or {}))
    import concourse.bass as bass
    import concourse.tile as tile
    from concourse import mybir

    f32 = mybir.dt.float32
    mdt = getattr(mybir.dt, mm_dtype_name)  # dtype of all matmul operands
    Relu = mybir.ActivationFunctionType.Relu
    Exp = mybir.ActivationFunctionType.Exp
    Alu = mybir.AluOpType

    nc = bass.Bass()
    x1t = nc.declare_dram_parameter("x1t", [IN1, B_CORE], mdt, isOutput=False)
    x2t = nc.declare_dram_parameter("x2t", [IN2, B_CORE], mdt, isOutput=False)
    w1t = nc.declare_dram_parameter("w1t", [4, 128, 128], mdt, isOutput=False)
    w2t = nc.declare_dram_parameter("w2t", [4, 128, 128], mdt, isOutput=False)
    woutt = nc.declare_dram_parameter("woutt", [4, 128, 128], mdt, isOutput=False)
    eu = nc.declare_dram_parameter("eu", [2, 128, 128], mdt, isOutput=False)
    ev = nc.declare_dram_parameter("ev", [2, 128, 128], mdt, isOutput=False)
    wc = nc.declare_dram_parameter("wc", [2, 128, 128], mdt, isOutput=False)
    suc = nc.declare_dram_parameter("suc", [128, 2], f32, isOutput=False)
    svc = nc.declare_dram_parameter("svc", [128, 2], f32, isOutput=False)
    bout = nc.declare_dram_parameter("bout", [128, 4], f32, isOutput=False)
    outt = nc.declare_dram_parameter("outt", [OUT, B_CORE], f32, isOutput=True)

    with tile.TileContext(nc) as tc:
        with (
            tc.tile_pool(name="consts", bufs=1) as consts,
            tc.tile_pool(name="inp", bufs=cfg["bufs_in"]) as pool_in,
            tc.tile_pool(name="tmp", bufs=2) as pool_tmp,
            tc.tile_pool(name="act", bufs=2) as pool_a,
            tc.tile_pool(name="tmul", bufs=cfg["bufs_t"]) as pool_t,
            tc.tile_pool(name="yep", bufs=2) as pool_ye,
            tc.tile_pool(name="outs", bufs=cfg["bufs_out"]) as pool_out,
            tc.tile_pool(name="pfc", bufs=cfg["bufs_pf"], space="PSUM") as pool_pf,
            tc.tile_pool(name="prep", bufs=cfg["bufs_pr"], space="PSUM") as pool_pr,
            tc.tile_pool(name="py", bufs=cfg["bufs_py"], space="PSUM") as pool_py,
            tc.tile_pool(name="po", bufs=cfg["bufs_po"], space="PSUM") as pool_po,
        ):
            # --- constants into SBUF ---
            def load_const3(h, nchunk=4):
                t = consts.tile([128, nchunk, 128], mdt, tag=h.name)
                nc.sync.dma_start(out=t, in_=h.rearrange("c k m -> k c m"))
                return t

            w1sb = load_const3(w1t)
            w2sb = load_const3(w2t)
            woutsb = load_const3(woutt)
            eusb = load_const3(eu, 2)
            evsb = load_const3(ev, 2)
            wcsb = load_const3(wc, 2)
            susb = consts.tile([128, 2], f32, tag="suc")
            nc.sync.dma_start(out=susb, in_=suc[:, :])
            svsb = consts.tile([128, 2], f32, tag="svc")
            nc.sync.dma_start(out=svsb, in_=svc[:, :])
            boutsb = consts.tile([128, 4], f32, tag="bout")
            nc.sync.dma_start(out=boutsb, in_=bout[:, :])

            x1v = x1t.rearrange("(c k) b -> k c b", k=128)
            x2v = x2t.rearrange("(c k) b -> k c b", k=128)
            outv = outt.rearrange("(c k) b -> k c b", k=128)

            import contextlib

            loop_cm = (
                tc.For_i(0, loop_reps, 1)
                if loop_reps is not None
                else contextlib.nullcontext()
            )
            with loop_cm:
                _batch_sweep(
                    nc, tc, reps, f32, mdt, Relu, Exp, Alu,
                    x1v, x2v, outv,
                    w1sb, w2sb, woutsb, eusb, evsb, wcsb,
                    susb, svsb, boutsb,
                    pool_in, pool_tmp, pool_a, pool_t, pool_ye, pool_out,
                    pool_pf, pool_pr, pool_py, pool_po, cfg,
                )

    _split_excess_waits(nc)
    return nc


def _batch_sweep(
    nc, tc, reps, f32, mdt, Relu, Exp, Alu,
    x1v, x2v, outv,
    w1sb, w2sb, woutsb, eusb, evsb, wcsb,
    susb, svsb, boutsb,
    pool_in, pool_tmp, pool_a, pool_t, pool_ye, pool_out,
    pool_pf, pool_pr, pool_py, pool_po, cfg,
):
    import concourse.bass as bass

    DB = cfg["db"]  # batch tiles per DMA transfer
    compute_only = cfg.get("compute_only", False)  # timing diagnostic
    x1_fix = x2_fix = None
    n_sup = N_TILES // DB
    for sup in range(n_sup * reps):
        sup = sup % n_sup
        bsup = bass.ds(sup * DB * NT, DB * NT)
        if compute_only and x1_fix is not None:
            x1sb, x2sb = x1_fix, x2_fix
        else:
            x1sb = pool_in.tile([128, 4, DB * NT], mdt, tag="x1")
            nc.sync.dma_start(out=x1sb, in_=x1v[:, :, bsup])
            x2sb = pool_in.tile([128, 4, DB * NT], mdt, tag="x2")
            nc.sync.dma_start(out=x2sb, in_=x2v[:, :, bsup])
            if compute_only:
                x1_fix, x2_fix = x1sb, x2sb
        outsb = pool_out.tile([128, 4, DB * NT], f32, tag="osb")

        yes = []
        for sub in range(DB):
            # --- fc1 / fc2 into one 2-bank PSUM tile ---
            p12 = pool_pf.tile([128, 2 * NT], f32, tag="pf")
            for half, (xsb, wsb) in enumerate(((x1sb, w1sb), (x2sb, w2sb))):
                for k in range(4):
                    nc.tensor.matmul(
                        p12[:, half * NT : (half + 1) * NT],
                        lhsT=wsb[:, k, :],
                        rhs=xsb[:, k, sub * NT : (sub + 1) * NT],
                        start=(k == 0),
                        stop=(k == 3),
                    )
            # ELU'+1 = relu(x) + min(exp(x), 1)   (x <= ~6 here, exp is safe)
            e12 = pool_tmp.tile([128, 2 * NT], f32, tag="e12")
            nc.scalar.activation(e12, p12, Exp)
            m12 = pool_tmp.tile([128, 2 * NT], f32, tag="m12")
            nc.vector.tensor_scalar_min(m12, e12, 1.0)
            a12 = pool_a.tile([128, 2 * NT], mdt, tag="a12")
            nc.vector.scalar_tensor_tensor(
                a12, in0=p12, scalar=0.0, in1=m12, op0=Alu.max, op1=Alu.add
            )
            a1 = a12[:, :NT]
            a2 = a12[:, NT:]

            # --- CP-space products: pr[(g,r)] = (U.x1-su)(V.x2-sv), 2 chunks
            yp = pool_py.tile([128, NT], f32, tag="py")
            pr_seq = cfg.get("pr_seq", False)
            for p in range(2):
                ru = pool_pr.tile([128, NT], f32, tag="pru")
                nc.tensor.matmul(ru, lhsT=eusb[:, p, :], rhs=a1)
                # stage (ru - su) into SBUF, then multiply by (rv - sv):
                # DVE reads at most one PSUM operand per op.
                ttf = pool_t.tile([128, NT], f32, tag="ttf")
                nc.vector.tensor_scalar(
                    ttf, ru, susb[:, p : p + 1], None, Alu.subtract
                )
                rv = pool_pr.tile(
                    [128, NT], f32, tag=("pru" if pr_seq else "prv")
                )
                nc.tensor.matmul(rv, lhsT=evsb[:, p, :], rhs=a2)
                tt = pool_t.tile([128, NT], mdt, tag="tt")
                nc.vector.scalar_tensor_tensor(
                    tt, in0=rv, scalar=svsb[:, p : p + 1], in1=ttf,
                    op0=Alu.subtract, op1=Alu.mult,
                )
                nc.tensor.matmul(
                    yp, lhsT=wcsb[:, p, :], rhs=tt,
                    start=(p == 0), stop=(p == 1),
                )

            # --- ELU' on y: ye = ELU(y) + 1 (relu(-x) form, y can be large) ---
            ny = pool_tmp.tile([128, NT], f32, tag="ny")
            nc.scalar.activation(ny, yp, Relu, scale=-1.0)
            ey = pool_tmp.tile([128, NT], f32, tag="ey")
            nc.scalar.activation(ey, ny, Exp, scale=-1.0)
            ye = pool_ye.tile([128, NT], mdt, tag="ye")
            nc.vector.scalar_tensor_tensor(
                ye, in0=yp, scalar=0.0, in1=ey, op0=Alu.max, op1=Alu.add
            )
            yes.append(ye)

        # --- fc_out (bias folds the ELU+1 offset) ---
        po_mode = cfg.get("po_mode", "merged")
        if po_mode == "merged":
            for c in range(4):
                o = pool_po.tile([128, DB * NT], f32, tag="po")
                for sub in range(DB):
                    ys = slice(sub * NT, (sub + 1) * NT)
                    nc.tensor.matmul(o[:, ys], lhsT=woutsb[:, c, :], rhs=yes[sub])
                nc.scalar.add(outsb[:, c, :], o, boutsb[:, c : c + 1])
        else:
            # per-sub [512] psum tiles; "share_pru" draws them from the pru tag
            pool = pool_pr if po_mode == "share_pru" else pool_po
            tag = "pru" if po_mode == "share_pru" else "po"
            for sub in range(DB):
                ys = slice(sub * NT, (sub + 1) * NT)
                for c in range(4):
                    o = pool.tile([128, NT], f32, tag=tag)
                    nc.tensor.matmul(o, lhsT=woutsb[:, c, :], rhs=yes[sub])
                    nc.scalar.add(outsb[:, c, ys], o, boutsb[:, c : c + 1])

        if not compute_only or sup == n_sup - 1:
            nc.sync.dma_start(out=outv[:, :, bsup], in_=outsb)


def _cp_decompose(Wb, R=8, seeds=8, iters=2500, target=1e-11):
    """Batched CP-ALS over all 32 groups: Wb[g,o,i,j] = sum_r C[g,o,r] U[g,i,r] V[g,j,r].
    Deterministic (fixed seeds). Returns (C, U, V, max_rel_err)."""
    T = Wb.astype(np.float64)  # [G, O, I, J]
    G, O, I, J = T.shape
    normT = np.linalg.norm(T.reshape(G, -1), axis=1)  # [G]

    bestC = np.zeros((G, O, R))
    bestU = np.zeros((G, I, R))
    bestV = np.zeros((G, J, R))
    best_err = np.full(G, np.inf)

    def solve(Tmat, KR):
        # Tmat [G, D, IJ], KR [G, IJ, R] -> X [G, D, R] minimizing ||Tmat - X KR^T||
        Gm = KR.transpose(0, 2, 1) @ KR  # [G, R, R]
        Gm = Gm + 1e-13 * np.eye(R)[None]
        rhs = Tmat @ KR  # [G, D, R]
        return np.linalg.solve(Gm, rhs.transpose(0, 2, 1)).transpose(0, 2, 1)

    for seed in range(seeds):
        active = best_err > target
        if not active.any():
            break
        rng = np.random.default_rng(1234 + seed)
        C = rng.standard_normal((G, O, R))
        U = rng.standard_normal((G, I, R))
        V = rng.standard_normal((G, J, R))
        for _ in range(iters):
            KR = (U[:, :, None, :] * V[:, None, :, :]).reshape(G, I * J, R)
            C = solve(T.reshape(G, O, I * J), KR)
            KR = (C[:, :, None, :] * V[:, None, :, :]).reshape(G, O * J, R)
            U = solve(T.transpose(0, 2, 1, 3).reshape(G, I, O * J), KR)
            KR = (C[:, :, None, :] * U[:, None, :, :]).reshape(G, O * I, R)
            V = solve(T.transpose(0, 3, 1, 2).reshape(G, J, O * I), KR)
            nc_ = np.linalg.norm(C, axis=1)
            nu = np.linalg.norm(U, axis=1)
            nv = np.linalg.norm(V, axis=1)
            s = (nc_ * nu * nv) ** (1.0 / 3.0)
            C *= (s / np.maximum(nc_, 1e-300))[:, None, :]
            U *= (s / np.maximum(nu, 1e-300))[:, None, :]
            V *= (s / np.maximum(nv, 1e-300))[:, None, :]
        rec = np.einsum("gor,gir,gjr->goij", C, U, V)
        err = np.linalg.norm((rec - T).reshape(G, -1), axis=1) / normT
        take = (err < best_err) & active
        bestC[take], bestU[take], bestV[take] = C[take], U[take], V[take]
        best_err[take] = err[take]
    return bestC, bestU, bestV, float(best_err.max())


def _make_consts(W1, W2, Wout, Wb):
    """Host-side constant matrices for the device program."""
    f = np.float32
    # lhsT chunks for fc1/fc2: [K=feat chunk, M=internal]
    w1t = np.stack([W1[:, k * 128 : (k + 1) * 128].T for k in range(4)]).astype(f)
    w2t = np.stack([W2[:, k * 128 : (k + 1) * 128].T for k in range(4)]).astype(f)
    # lhsT chunks for fc_out: [K=internal, M=out chunk]
    woutt = np.stack([Wout[c * 128 : (c + 1) * 128, :].T for c in range(4)]).astype(f)

    # CP decomposition of the per-group bilinear tensors (R=8 is exact for
    # generic 4x4x4): y[g,o] = sum_r C[g,o,r] (U[g,:,r].x1g) (V[g,:,r].x2g)
    R = 8
    C, U, V, cp_err = _cp_decompose(Wb, R=R)
    # CP-space row layout: chunk p (<2) holds groups [16p, 16p+16), row
    # m = g_loc*8 + r.
    eu = np.zeros((2, 128, 128), f)  # lhsT: [k=(g*4+i), m=(g_loc*8+r)]
    ev = np.zeros((2, 128, 128), f)
    wcm = np.zeros((2, 128, 128), f)  # lhsT: [k=(g_loc*8+r), m=(g*4+o)]
    suc = np.zeros((128, 2), f)
    svc = np.zeros((128, 2), f)
    for p in range(2):
        for gl in range(16):
            g = p * 16 + gl
            for r in range(R):
                m = gl * 8 + r
                eu[p, g * 4 : g * 4 + 4, m] = U[g, :, r]
                ev[p, g * 4 : g * 4 + 4, m] = V[g, :, r]
                wcm[p, m, g * 4 : g * 4 + 4] = C[g, :, r]
                suc[m, p] = U[g, :, r].sum()
                svc[m, p] = V[g, :, r].sum()

    # fc_out bias: out = Wout @ (ye' - 1) = Wout@ye' - Wout@1
    bvec = -Wout.astype(np.float64).sum(axis=1)
    bout = np.stack([bvec[c * 128 : (c + 1) * 128] for c in range(4)], axis=1).astype(f)

    return dict(
        w1t=w1t, w2t=w2t, woutt=woutt, eu=eu, ev=ev, wc=wcm,
        suc=suc, svc=svc, bout=bout,
    )


def kernel(input1, input2, W1, W2, Wout, Wb):
    _ensure_path()
    from concourse.bass_utils import run_bass_kernel_spmd

    if "nc" not in _CACHE:
        _CACHE["nc"] = _build_program()
    nc = _CACHE["nc"]

    W1, W2, Wout, Wb = (np.asarray(a) for a in (W1, W2, Wout, Wb))
    ckey = (W1.tobytes()[:64], Wb.tobytes()[:256])
    if _CACHE.get("ckey") != ckey:
        _CACHE["consts"] = _make_consts(W1, W2, Wout, Wb)
        _CACHE["ckey"] = ckey
    consts = _CACHE["consts"]
    input1 = np.asarray(input1)
    input2 = np.asarray(input2)

    in_maps = []
    for c in range(N_CORES):
        sl = slice(c * B_CORE, (c + 1) * B_CORE)
        m = dict(consts)
        m["x1t"] = np.ascontiguousarray(input1[sl].T)
        m["x2t"] = np.ascontiguousarray(input2[sl].T)
        in_maps.append(m)

    res = run_bass_kernel_spmd(nc, in_maps, list(range(N_CORES)))
    _CACHE["last_result"] = res

    out = np.empty((B, OUT), np.float32)
    for c in range(N_CORES):
        out[c * B_CORE : (c + 1) * B_CORE, :] = res.results[c]["outt"].T
    return out



# revision 7
# speedup vs baseline: 1.4437x; 1.4437x over previous
"""Trainium2 Bass kernel for nn_AggregatedBilinear.

Computation (per batch row b):
    x1 = ELU(input1 @ W1.T)                    # [128]
    x2 = ELU(input2 @ W2.T)                    # [128]
    y[g,o] = sum_ij x1[g,i] Wb[g,o,i,j] x2[g,j]   (g<32, o,i,j<4)
    out = ELU(y) @ Wout.T                      # [512]

Strategy: data-parallel over 8 NeuronCores (8192 batch rows each). On-chip
layout is feature-major (features on SBUF partitions, batch on the free dim),
so the host pre-transposes each input shard (fp32 DMA-transpose does not
exist on trn2) and transposes the result back.

The per-group bilinear is computed as
    t'[(g,i,j)] = (E1 @ A1') * (E2 @ A2')       # PE expand + DVE multiply
    Y = Wblk.T @ t' - C1 @ A1' - C2 @ A2'       # PE, block-diagonal consts
where A' = ELU(x)+1 = relu(x) + exp(min(x,0)) (computed with 2 ACT ops + one
fused DVE scalar_tensor_tensor), and the -1 corrections are folded into the
constant matrices C1[(g,o),(g,i)] = sum_j Wb[g,o,i,j], C2, and per-partition
bias vectors (c0 = sum_ij Wb, bout = -Wout @ (1 - c0)).
"""

import numpy as np

B = 65536
IN1 = IN2 = 512
OUT = 512
CARD = 32
WIDTH = 4
INTERNAL = CARD * WIDTH  # 128
N_CORES = 8
B_CORE = B // N_CORES  # 8192
NT = 512  # batch columns per tile (one PSUM bank)
N_TILES = B_CORE // NT  # 16

_CACHE = {}


def _ensure_path():
    import sys

    try:
        import concourse  # noqa: F401
    except ImportError:
        for p in ("/opt/trn_rl_repo", "/root/.axon_site/_ro/trn_rl_repo"):
            if p not in sys.path:
                sys.path.insert(0, p)


def _split_excess_waits(nc, max_waits=1):
    """walrus CoreV3 codegen rejects instructions with more than a couple of
    semaphore waits; split excess waits onto preceding NoOps."""
    from concourse import mybir

    n_new = 0
    for f in nc.m.functions:
        for bb in f.blocks:
            insts = list(bb.instructions)
            out = []
            changed = False
            for inst in insts:
                si = inst.sync_info
                if si is not None and si.on_wait and len(si.on_wait) > max_waits:
                    waits = list(si.on_wait)
                    excess, keep = waits[:-max_waits], waits[-max_waits:]
                    for i in range(0, len(excess), max_waits):
                        nop = mybir.InstNoOp(
                            name=f"waitsplit-{n_new}",
                            engine=inst.engine,
                            ins=[],
                            outs=[],
                            sync_info=mybir.SyncInfo(
                                on_wait=excess[i : i + max_waits], on_update=[]
                            ),
                        )
                        n_new += 1
                        out.append(nop)
                    inst.sync_info = mybir.SyncInfo(
                        on_wait=keep, on_update=list(si.on_update or [])
                    )
                    changed = True
                out.append(inst)
            if changed:
                bb.instructions[:] = out
    return n_new


DEFAULT_CFG = dict(db=2, bufs_in=2, bufs_pf=1, bufs_pr=1, bufs_py=2, bufs_po=1,
                   bufs_t=4, bufs_out=2)


def _build_program(mm_dtype_name="float32r", reps=1, loop_reps=None, cfg=None):
    """loop_reps: wrap the whole batch sweep in an on-device For_i that runs
    it loop_reps times (same I/O; used only for timing measurements)."""
    cfg = dict(DEFAULT_CFG, **(cfg or {}))
    import concourse.bass as bass
    import concourse.tile as tile
    from concourse import mybir

    f32 = mybir.dt.float32
    mdt = getattr(mybir.dt, mm_dtype_name)  # dtype of all matmul operands
    Relu = mybir.ActivationFunctionType.Relu
    Exp = mybir.ActivationFunctionType.Exp
    Alu = mybir.AluOpType

    odt = getattr(mybir.dt, cfg.get("out_dtype_name", "float32"))
    nc = bass.Bass()
    x1t = nc.declare_dram_parameter("x1t", [IN1, B_CORE], mdt, isOutput=False)
    x2t = nc.declare_dram_parameter("x2t", [IN2, B_CORE], mdt, isOutput=False)
    w1t = nc.declare_dram_parameter("w1t", [4, 128, 128], mdt, isOutput=False)
    w2t = nc.declare_dram_parameter("w2t", [4, 128, 128], mdt, isOutput=False)
    woutt = nc.declare_dram_parameter("woutt", [4, 128, 128], mdt, isOutput=False)
    eu = nc.declare_dram_parameter("eu", [2, 128, 128], mdt, isOutput=False)
    ev = nc.declare_dram_parameter("ev", [2, 128, 128], mdt, isOutput=False)
    wc = nc.declare_dram_parameter("wc", [2, 128, 128], mdt, isOutput=False)
    suc = nc.declare_dram_parameter("suc", [128, 2], f32, isOutput=False)
    svc = nc.declare_dram_parameter("svc", [128, 2], f32, isOutput=False)
    bout = nc.declare_dram_parameter("bout", [128, 4], f32, isOutput=False)
    outt = nc.declare_dram_parameter("outt", [OUT, B_CORE], odt, isOutput=True)

    with tile.TileContext(nc) as tc:
        with (
            tc.tile_pool(name="consts", bufs=1) as consts,
            tc.tile_pool(name="inp", bufs=cfg["bufs_in"]) as pool_in,
            tc.tile_pool(name="tmp", bufs=2) as pool_tmp,
            tc.tile_pool(name="act", bufs=2) as pool_a,
            tc.tile_pool(name="tmul", bufs=cfg["bufs_t"]) as pool_t,
            tc.tile_pool(name="yep", bufs=2) as pool_ye,
            tc.tile_pool(name="outs", bufs=cfg["bufs_out"]) as pool_out,
            tc.tile_pool(name="pfc", bufs=cfg["bufs_pf"], space="PSUM") as pool_pf,
            tc.tile_pool(name="prep", bufs=cfg["bufs_pr"], space="PSUM") as pool_pr,
            tc.tile_pool(name="py", bufs=cfg["bufs_py"], space="PSUM") as pool_py,
            tc.tile_pool(name="po", bufs=cfg["bufs_po"], space="PSUM") as pool_po,
        ):
            # --- constants into SBUF ---
            def load_const3(h, nchunk=4):
                t = consts.tile([128, nchunk, 128], mdt, tag=h.name)
                nc.sync.dma_start(out=t, in_=h.rearrange("c k m -> k c m"))
                return t

            w1sb = load_const3(w1t)
            w2sb = load_const3(w2t)
            woutsb = load_const3(woutt)
            eusb = load_const3(eu, 2)
            evsb = load_const3(ev, 2)
            wcsb = load_const3(wc, 2)
            susb = consts.tile([128, 2], f32, tag="suc")
            nc.sync.dma_start(out=susb, in_=suc[:, :])
            svsb = consts.tile([128, 2], f32, tag="svc")
            nc.sync.dma_start(out=svsb, in_=svc[:, :])
            boutsb = consts.tile([128, 4], f32, tag="bout")
            nc.sync.dma_start(out=boutsb, in_=bout[:, :])

            x1v = x1t.rearrange("(c k) b -> k c b", k=128)
            x2v = x2t.rearrange("(c k) b -> k c b", k=128)
            outv = outt.rearrange("(c k) b -> k c b", k=128)

            import contextlib

            loop_cm = (
                tc.For_i(0, loop_reps, 1)
                if loop_reps is not None
                else contextlib.nullcontext()
            )
            with loop_cm:
                _batch_sweep(
                    nc, tc, reps, f32, mdt, odt, Relu, Exp, Alu,
                    x1v, x2v, outv,
                    w1sb, w2sb, woutsb, eusb, evsb, wcsb,
                    susb, svsb, boutsb,
                    pool_in, pool_tmp, pool_a, pool_t, pool_ye, pool_out,
                    pool_pf, pool_pr, pool_py, pool_po, cfg,
                )

    _split_excess_waits(nc)
    return nc


def _batch_sweep(
    nc, tc, reps, f32, mdt, odt, Relu, Exp, Alu,
    x1v, x2v, outv,
    w1sb, w2sb, woutsb, eusb, evsb, wcsb,
    susb, svsb, boutsb,
    pool_in, pool_tmp, pool_a, pool_t, pool_ye, pool_out,
    pool_pf, pool_pr, pool_py, pool_po, cfg,
):
    import concourse.bass as bass

    DB = cfg["db"]  # batch tiles per DMA transfer
    compute_only = cfg.get("compute_only", False)  # timing diagnostic
    x1_fix = x2_fix = None
    n_sup = N_TILES // DB
    for sup in range(n_sup * reps):
        sup = sup % n_sup
        bsup = bass.ds(sup * DB * NT, DB * NT)
        if compute_only and x1_fix is not None:
            x1sb, x2sb = x1_fix, x2_fix
        else:
            x1sb = pool_in.tile([128, 4, DB * NT], mdt, tag="x1")
            nc.sync.dma_start(out=x1sb, in_=x1v[:, :, bsup])
            x2sb = pool_in.tile([128, 4, DB * NT], mdt, tag="x2")
            nc.sync.dma_start(out=x2sb, in_=x2v[:, :, bsup])
            if compute_only:
                x1_fix, x2_fix = x1sb, x2sb
        outsb = pool_out.tile([128, 4, DB * NT], odt, tag="osb")

        yes = []
        for sub in range(DB):
            # --- fc1 / fc2 into one 2-bank PSUM tile ---
            p12 = pool_pf.tile([128, 2 * NT], f32, tag="pf")
            for half, (xsb, wsb) in enumerate(((x1sb, w1sb), (x2sb, w2sb))):
                for k in range(4):
                    nc.tensor.matmul(
                        p12[:, half * NT : (half + 1) * NT],
                        lhsT=wsb[:, k, :],
                        rhs=xsb[:, k, sub * NT : (sub + 1) * NT],
                        start=(k == 0),
                        stop=(k == 3),
                    )
            # ELU'+1 = relu(x) + min(exp(x), 1)   (x <= ~6 here, exp is safe)
            e12 = pool_tmp.tile([128, 2 * NT], f32, tag="e12")
            nc.scalar.activation(e12, p12, Exp)
            m12 = pool_tmp.tile([128, 2 * NT], f32, tag="m12")
            nc.vector.tensor_scalar_min(m12, e12, 1.0)
            a12 = pool_a.tile([128, 2 * NT], mdt, tag="a12")
            nc.vector.scalar_tensor_tensor(
                a12, in0=p12, scalar=0.0, in1=m12, op0=Alu.max, op1=Alu.add
            )
            a1 = a12[:, :NT]
            a2 = a12[:, NT:]

            # --- CP-space products: pr[(g,r)] = (U.x1-su)(V.x2-sv), 2 chunks
            yp = pool_py.tile([128, NT], f32, tag="py")
            pr_seq = cfg.get("pr_seq", False)
            for p in range(2):
                ru = pool_pr.tile([128, NT], f32, tag="pru")
                nc.tensor.matmul(ru, lhsT=eusb[:, p, :], rhs=a1)
                # stage (ru - su) into SBUF, then multiply by (rv - sv):
                # DVE reads at most one PSUM operand per op.
                ttf = pool_t.tile([128, NT], f32, tag="ttf")
                nc.vector.tensor_scalar(
                    ttf, ru, susb[:, p : p + 1], None, Alu.subtract
                )
                rv = pool_pr.tile(
                    [128, NT], f32, tag=("pru" if pr_seq else "prv")
                )
                nc.tensor.matmul(rv, lhsT=evsb[:, p, :], rhs=a2)
                tt = pool_t.tile([128, NT], mdt, tag="tt")
                nc.vector.scalar_tensor_tensor(
                    tt, in0=rv, scalar=svsb[:, p : p + 1], in1=ttf,
                    op0=Alu.subtract, op1=Alu.mult,
                )
                nc.tensor.matmul(
                    yp, lhsT=wcsb[:, p, :], rhs=tt,
                    start=(p == 0), stop=(p == 1),
                )

            # --- ELU' on y: ye = ELU(y) + 1 (relu(-x) form, y can be large) ---
            ny = pool_tmp.tile([128, NT], f32, tag="ny")
            nc.scalar.activation(ny, yp, Relu, scale=-1.0)
            ey = pool_tmp.tile([128, NT], f32, tag="ey")
            nc.scalar.activation(ey, ny, Exp, scale=-1.0)
            ye = pool_ye.tile([128, NT], mdt, tag="ye")
            nc.vector.scalar_tensor_tensor(
                ye, in0=yp, scalar=0.0, in1=ey, op0=Alu.max, op1=Alu.add
            )
            yes.append(ye)

        # --- fc_out (bias folds the ELU+1 offset) ---
        po_mode = cfg.get("po_mode", "merged")
        if po_mode == "merged":
            for c in range(4):
                o = pool_po.tile([128, DB * NT], f32, tag="po")
                for sub in range(DB):
                    ys = slice(sub * NT, (sub + 1) * NT)
                    nc.tensor.matmul(o[:, ys], lhsT=woutsb[:, c, :], rhs=yes[sub])
                nc.scalar.add(outsb[:, c, :], o, boutsb[:, c : c + 1])
        else:
            # per-sub [512] psum tiles; "share_pru" draws them from the pru tag
            pool = pool_pr if po_mode == "share_pru" else pool_po
            tag = "pru" if po_mode == "share_pru" else "po"
            for sub in range(DB):
                ys = slice(sub * NT, (sub + 1) * NT)
                for c in range(4):
                    o = pool.tile([128, NT], f32, tag=tag)
                    nc.tensor.matmul(o, lhsT=woutsb[:, c, :], rhs=yes[sub])
                    nc.scalar.add(outsb[:, c, ys], o, boutsb[:, c : c + 1])

        if not compute_only or sup == n_sup - 1:
            nc.sync.dma_start(out=outv[:, :, bsup], in_=outsb)


def _cp_decompose(Wb, R=8, seeds=8, iters=2500, target=1e-11):
    """Batched CP-ALS over all 32 groups: Wb[g,o,i,j] = sum_r C[g,o,r] U[g,i,r] V[g,j,r].
    Deterministic (fixed seeds). Returns (C, U, V, max_rel_err)."""
    T = Wb.astype(np.float64)  # [G, O, I, J]
    G, O, I, J = T.shape
    normT = np.linalg.norm(T.reshape(G, -1), axis=1)  # [G]

    bestC = np.zeros((G, O, R))
    bestU = np.zeros((G, I, R))
    bestV = np.zeros((G, J, R))
    best_err = np.full(G, np.inf)

    def solve(Tmat, KR):
        # Tmat [G, D, IJ], KR [G, IJ, R] -> X [G, D, R] minimizing ||Tmat - X KR^T||
        Gm = KR.transpose(0, 2, 1) @ KR  # [G, R, R]
        Gm = Gm + 1e-13 * np.eye(R)[None]
        rhs = Tmat @ KR  # [G, D, R]
        return np.linalg.solve(Gm, rhs.transpose(0, 2, 1)).transpose(0, 2, 1)

    for seed in range(seeds):
        active = best_err > target
        if not active.any():
            break
        rng = np.random.default_rng(1234 + seed)
        C = rng.standard_normal((G, O, R))
        U = rng.standard_normal((G, I, R))
        V = rng.standard_normal((G, J, R))
        for _ in range(iters):
            KR = (U[:, :, None, :] * V[:, None, :, :]).reshape(G, I * J, R)
            C = solve(T.reshape(G, O, I * J), KR)
            KR = (C[:, :, None, :] * V[:, None, :, :]).reshape(G, O * J, R)
            U = solve(T.transpose(0, 2, 1, 3).reshape(G, I, O * J), KR)
            KR = (C[:, :, None, :] * U[:, None, :, :]).reshape(G, O * I, R)
            V = solve(T.transpose(0, 3, 1, 2).reshape(G, J, O * I), KR)
            nc_ = np.linalg.norm(C, axis=1)
            nu = np.linalg.norm(U, axis=1)
            nv = np.linalg.norm(V, axis=1)
            s = (nc_ * nu * nv) ** (1.0 / 3.0)
            C *= (s / np.maximum(nc_, 1e-300))[:, None, :]
            U *= (s / np.maximum(nu, 1e-300))[:, None, :]
            V *= (s / np.maximum(nv, 1e-300))[:, None, :]
        rec = np.einsum("gor,gir,gjr->goij", C, U, V)
        err = np.linalg.norm((rec - T).reshape(G, -1), axis=1) / normT
        take = (err < best_err) & active
        bestC[take], bestU[take], bestV[take] = C[take], U[take], V[take]
        best_err[take] = err[take]
    return bestC, bestU, bestV, float(best_err.max())


def _make_consts(W1, W2, Wout, Wb):
    """Host-side constant matrices for the device program."""
    f = np.float32
    # lhsT chunks for fc1/fc2: [K=feat chunk, M=internal]
    w1t = np.stack([W1[:, k * 128 : (k + 1) * 128].T for k in range(4)]).astype(f)
    w2t = np.stack([W2[:, k * 128 : (k + 1) * 128].T for k in range(4)]).astype(f)
    # lhsT chunks for fc_out: [K=internal, M=out chunk]
    woutt = np.stack([Wout[c * 128 : (c + 1) * 128, :].T for c in range(4)]).astype(f)

    # CP decomposition of the per-group bilinear tensors (R=8 is exact for
    # generic 4x4x4): y[g,o] = sum_r C[g,o,r] (U[g,:,r].x1g) (V[g,:,r].x2g)
    R = 8
    C, U, V, cp_err = _cp_decompose(Wb, R=R)
    # CP-space row layout: chunk p (<2) holds groups [16p, 16p+16), row
    # m = g_loc*8 + r.
    eu = np.zeros((2, 128, 128), f)  # lhsT: [k=(g*4+i), m=(g_loc*8+r)]
    ev = np.zeros((2, 128, 128), f)
    wcm = np.zeros((2, 128, 128), f)  # lhsT: [k=(g_loc*8+r), m=(g*4+o)]
    suc = np.zeros((128, 2), f)
    svc = np.zeros((128, 2), f)
    for p in range(2):
        for gl in range(16):
            g = p * 16 + gl
            for r in range(R):
                m = gl * 8 + r
                eu[p, g * 4 : g * 4 + 4, m] = U[g, :, r]
                ev[p, g * 4 : g * 4 + 4, m] = V[g, :, r]
                wcm[p, m, g * 4 : g * 4 + 4] = C[g, :, r]
                suc[m, p] = U[g, :, r].sum()
                svc[m, p] = V[g, :, r].sum()

    # fc_out bias: out = Wout @ (ye' - 1) = Wout@ye' - Wout@1
    bvec = -Wout.astype(np.float64).sum(axis=1)
    bout = np.stack([bvec[c * 128 : (c + 1) * 128] for c in range(4)], axis=1).astype(f)

    return dict(
        w1t=w1t, w2t=w2t, woutt=woutt, eu=eu, ev=ev, wc=wcm,
        suc=suc, svc=svc, bout=bout,
    )


MM_DTYPE = "float16"  # dtype of matmul operands + DMA'd inputs
OUT_DTYPE = "float16"  # dtype the result leaves the device in

_NP_DT = {"float16": np.float16, "float32": np.float32, "float32r": np.float32}


def kernel(input1, input2, W1, W2, Wout, Wb):
    _ensure_path()
    from concourse.bass_utils import run_bass_kernel_spmd

    if "nc" not in _CACHE:
        _CACHE["nc"] = _build_program(
            mm_dtype_name=MM_DTYPE, cfg=dict(out_dtype_name=OUT_DTYPE)
        )
    nc = _CACHE["nc"]
    np_mdt = _NP_DT[MM_DTYPE]

    W1, W2, Wout, Wb = (np.asarray(a) for a in (W1, W2, Wout, Wb))
    ckey = (W1.tobytes()[:64], Wb.tobytes()[:256])
    if _CACHE.get("ckey") != ckey:
        consts = _make_consts(W1, W2, Wout, Wb)
        for k in ("w1t", "w2t", "woutt", "eu", "ev", "wc"):
            consts[k] = consts[k].astype(np_mdt)
        _CACHE["consts"] = consts
        _CACHE["ckey"] = ckey
    consts = _CACHE["consts"]
    input1 = np.asarray(input1)
    input2 = np.asarray(input2)

    in_maps = []
    for c in range(N_CORES):
        sl = slice(c * B_CORE, (c + 1) * B_CORE)
        m = dict(consts)
        m["x1t"] = np.ascontiguousarray(input1[sl].T.astype(np_mdt))
        m["x2t"] = np.ascontiguousarray(input2[sl].T.astype(np_mdt))
        in_maps.append(m)

    res = run_bass_kernel_spmd(nc, in_maps, list(range(N_CORES)))
    _CACHE["last_result"] = res

    out = np.empty((B, OUT), np.float32)
    for c in range(N_CORES):
        out[c * B_CORE : (c + 1) * B_CORE, :] = res.results[c]["outt"].T
    return out

